# revision 13
# baseline (speedup 1.0000x reference)
"""Trainium2 Bass kernel for 12-head causal MHA (B=2, S=2048, D=768), fp32.

Sharding: 8 cores = (batch b in {0,1}) x (head-group hg in {0..3}, 3 heads each).
Each core computes, for its (b, hg):
    qT/kT = (x wq_hg^T)^T  (transposed layout, [192, S])
    v     = x wv_hg^T      (natural layout, + ones column for softmax denom)
    flash-style causal attention without max-subtraction (scores are O(1))
    partial yT = wo_hg^T @ outT   ([768, S], row-parallel partial)
Host sums the 4 head-group partials per batch, transposes, adds bo.

Matmul operands live in SBUF as float32r (fp32 bits; 1 cycle/row at N>=256).
The causal mask is a multiplicative 0/1 mask sliced from a [128, 1024]
sliding-window matrix (host input), applied only on diagonal-band tiles.
"""

import math
from contextlib import ExitStack

import numpy as np

import concourse.bacc as bacc
import concourse.bass as bass
import concourse.mybir as mybir
import concourse.tile as tile

FP32 = mybir.dt.float32
FP32R = mybir.dt.float32r

B = 2
S = 2048
D = 768
NH = 12
DK = 64
NCORES = 8
HG = 3  # heads per core
HD = HG * DK  # 192
VP = 256  # padded v width (>=256 keeps float32r at full rate)
KC = D // 128  # 6 contraction chunks of 128
SB = 512  # sequence block (matmul N)
NJ = S // SB  # 4
NT = S // 128  # 16 key tiles
SCALE = 1.0 / math.sqrt(DK)


def _r(ap):
    """float32r view of an fp32 DRAM AP (same bytes) for DMA into fp32r SBUF."""
    return ap.bitcast(FP32R)


def build_nc(causal: bool):
    nc = bacc.Bacc(trn_type="TRN2", target_bir_lowering=False, debug=False)

    xT_d = nc.declare_dram_parameter("xT", [D, S], FP32, isOutput=False)
    wqT_d = nc.declare_dram_parameter("wqT", [D, HD], FP32, isOutput=False)
    wkT_d = nc.declare_dram_parameter("wkT", [D, HD], FP32, isOutput=False)
    wvT_d = nc.declare_dram_parameter("wvT", [D, VP], FP32, isOutput=False)
    woT_d = nc.declare_dram_parameter("woT", [HD, D], FP32, isOutput=False)
    bq_d = nc.declare_dram_parameter("bq", [HD], FP32, isOutput=False)
    bk_d = nc.declare_dram_parameter("bk", [HD], FP32, isOutput=False)
    bv_d = nc.declare_dram_parameter("bv", [VP], FP32, isOutput=False)
    cm_d = nc.declare_dram_parameter("cmask", [128, 2 * SB], FP32, isOutput=False)
    yT_d = nc.declare_dram_parameter("yT", [D, S], FP32, isOutput=True)

    EXP = mybir.ActivationFunctionType.Exp

    with tile.TileContext(nc) as tc, ExitStack() as ctx:
        consts = ctx.enter_context(tc.tile_pool(name="consts", bufs=1))

        # ---- constant / persistent SBUF tensors ----
        xT_sb = consts.tile([128, KC, S], FP32R)  # x^T, chunk c = rows 128c..
        wqT_sb = consts.tile([128, KC, HD], FP32R)
        wkT_sb = consts.tile([128, KC, HD], FP32R)
        wvT_sb = consts.tile([128, KC, VP], FP32R)
        woT_sb = [consts.tile([64, D], FP32R, name=f"woT{h}") for h in range(HG)]
        bq0_sb = consts.tile([128, 1], FP32, name="bq0")
        bq1_sb = consts.tile([64, 1], FP32, name="bq1")
        bk0_sb = consts.tile([128, 1], FP32, name="bk0")
        bk1_sb = consts.tile([64, 1], FP32, name="bk1")
        bvb_sb = consts.tile([128, VP], FP32)  # bv broadcast to all partitions
        v65_sb = consts.tile([128, NT, HG, 65], FP32R)  # v tiles + ones column
        qT01_sb = consts.tile([128, S], FP32R)  # q^T heads 0,1
        qT2_sb = consts.tile([64, S], FP32R)  # q^T head 2
        kT01_sb = consts.tile([128, S], FP32R)
        kT2_sb = consts.tile([64, S], FP32R)
        if causal:
            cm_sb = consts.tile([128, 2 * SB], FP32)
            nc.sync.dma_start(out=cm_sb, in_=cm_d.ap())

        xT_r = xT_d.ap().rearrange("(c p) s -> p c s", p=128)
        for c in range(KC):
            nc.sync.dma_start(out=xT_sb[:, c, :], in_=_r(xT_r[:, c, :]))
        nc.sync.dma_start(
            out=wqT_sb, in_=_r(wqT_d.ap().rearrange("(c p) n -> p c n", p=128))
        )
        nc.sync.dma_start(
            out=wkT_sb, in_=_r(wkT_d.ap().rearrange("(c p) n -> p c n", p=128))
        )
        nc.sync.dma_start(
            out=wvT_sb, in_=_r(wvT_d.ap().rearrange("(c p) n -> p c n", p=128))
        )
        for h in range(HG):
            nc.sync.dma_start(
                out=woT_sb[h], in_=_r(woT_d.ap()[h * 64 : (h + 1) * 64, :])
            )
        nc.sync.dma_start(
            out=bq0_sb, in_=bq_d.ap()[0:128].rearrange("(p o) -> p o", o=1)
        )
        nc.sync.dma_start(
            out=bq1_sb, in_=bq_d.ap()[128:192].rearrange("(p o) -> p o", o=1)
        )
        nc.sync.dma_start(
            out=bk0_sb, in_=bk_d.ap()[0:128].rearrange("(p o) -> p o", o=1)
        )
        nc.sync.dma_start(
            out=bk1_sb, in_=bk_d.ap()[128:192].rearrange("(p o) -> p o", o=1)
        )
        # broadcast bv across all 128 partitions with a step-0 partition AP
        bv_ap = bv_d.ap()
        bvb_src = bass.AP(
            tensor=bv_ap.tensor, offset=bv_ap.offset, ap=[[0, 128], [1, VP]]
        )
        nc.sync.dma_start(out=bvb_sb, in_=bvb_src)

        nc.vector.memset(v65_sb.bitcast(FP32), 1.0)  # preset ones column

        # ---- phase 1: v projection, natural layout (x^T stationary) ----
        with tc.tile_pool(name="vp", bufs=2, space="PSUM") as vp_pool:
            for st in range(NT):
                vp = vp_pool.tile([128, VP], FP32)
                for c in range(KC):
                    nc.tensor.matmul(
                        vp,
                        lhsT=xT_sb[:, c, st * 128 : (st + 1) * 128],
                        rhs=wvT_sb[:, c, :],
                        start=(c == 0),
                        stop=(c == KC - 1),
                    )
                for h in range(HG):
                    nc.vector.tensor_add(
                        v65_sb[:, st, h, 0:64],
                        vp[:, h * 64 : (h + 1) * 64],
                        bvb_sb[:, h * 64 : (h + 1) * 64],
                    )

        # ---- phase 2: q/k projections, transposed layout (w stationary) ----
        with tc.tile_pool(name="pp", bufs=2, space="PSUM") as pp_pool:
            for w_sb, b0, b1, dst01, dst2 in (
                (wqT_sb, bq0_sb, bq1_sb, qT01_sb, qT2_sb),
                (wkT_sb, bk0_sb, bk1_sb, kT01_sb, kT2_sb),
            ):
                for mt, m, dst, bias in ((0, 128, dst01, b0), (1, 64, dst2, b1)):
                    for j in range(NJ):
                        pp = pp_pool.tile([128, SB], FP32)
                        for c in range(KC):
                            nc.tensor.matmul(
                                pp[0:m, :],
                                lhsT=w_sb[:, c, mt * 128 : mt * 128 + m],
                                rhs=xT_sb[:, c, j * SB : (j + 1) * SB],
                                start=(c == 0),
                                stop=(c == KC - 1),
                            )
                        nc.vector.tensor_scalar_add(
                            dst[0:m, j * SB : (j + 1) * SB], pp[0:m, :], bias[0:m, :]
                        )

        # ---- phase 3: attention + output projection ----
        sp_pool = ctx.enter_context(tc.tile_pool(name="sp", bufs=3, space="PSUM"))
        op_pool = ctx.enter_context(tc.tile_pool(name="op", bufs=2, space="PSUM"))
        yp_pool = ctx.enter_context(tc.tile_pool(name="yp", bufs=2, space="PSUM"))
        et_pool = ctx.enter_context(tc.tile_pool(name="et", bufs=3))
        ef_pool = ctx.enter_context(tc.tile_pool(name="ef", bufs=2))
        rc_pool = ctx.enter_context(tc.tile_pool(name="rc", bufs=2))
        ot_pool = ctx.enter_context(tc.tile_pool(name="ot", bufs=6))
        yt_pool = ctx.enter_context(tc.tile_pool(name="yt", bufs=3))

        for j in range(NJ):
            out_tiles = []
            for h in range(HG):
                if h < 2:
                    qsrc, ksrc, base = qT01_sb, kT01_sb, 64 * h
                else:
                    qsrc, ksrc, base = qT2_sb, kT2_sb, 0
                tend = 4 * (j + 1) if causal else NT
                op = op_pool.tile([65, SB], FP32)
                for t in range(tend):
                    sp = sp_pool.tile([128, SB], FP32)
                    nc.tensor.matmul(
                        sp,
                        lhsT=ksrc[base : base + 64, t * 128 : (t + 1) * 128],
                        rhs=qsrc[base : base + 64, j * SB : (j + 1) * SB],
                        start=True,
                        stop=True,
                    )
                    et = et_pool.tile([128, SB], FP32R)
                    if causal and t >= 4 * j:
                        # diagonal band: exp then multiply by 0/1 causal mask
                        # slice (keep iff 128t+p <= 512j+c)
                        ef = ef_pool.tile([128, SB], FP32)
                        nc.scalar.activation(ef, sp, EXP, scale=SCALE)
                        s0 = SB + SB * j - 128 * t
                        nc.vector.tensor_mul(et, ef, cm_sb[:, s0 : s0 + SB])
                    else:
                        nc.scalar.activation(et, sp, EXP, scale=SCALE)
                    nc.tensor.matmul(
                        op,
                        lhsT=v65_sb[:, t, h, :],
                        rhs=et,
                        start=(t == 0),
                        stop=(t == tend - 1),
                    )
                # normalize: rows 0:64 / row 64 (gpsimd partition broadcast).
                # partition_broadcast HW ucode reads partition 0 regardless of
                # the AP offset, so DMA-hop the reciprocal row to partition 0.
                rc = rc_pool.tile([65, SB], FP32)
                nc.vector.reciprocal(rc[64:65, :], op[64:65, :])
                rz = rc_pool.tile([1, SB], FP32, name="rz")
                nc.sync.dma_start(out=rz, in_=rc[64:65, :])
                bc = rc_pool.tile([64, SB], FP32, name="bc")
                nc.gpsimd.partition_broadcast(bc, rz[0:1, :])
                ot = ot_pool.tile([64, SB], FP32R)
                nc.vector.tensor_mul(ot, op[0:64, :], bc)
                out_tiles.append(ot)

            for dt in range(KC):
                yp = yp_pool.tile([128, SB], FP32)
                for h in range(HG):
                    nc.tensor.matmul(
                        yp,
                        lhsT=woT_sb[h][:, dt * 128 : (dt + 1) * 128],
                        rhs=out_tiles[h],
                        start=(h == 0),
                        stop=(h == HG - 1),
                    )
                yt = yt_pool.tile([128, SB], FP32)
                nc.scalar.copy(yt, yp)
                nc.sync.dma_start(
                    out=yT_d.ap()[dt * 128 : (dt + 1) * 128, j * SB : (j + 1) * SB],
                    in_=yt,
                )

    nc.finalize()
    return nc


_NC_CACHE: dict[bool, object] = {}


def get_nc(causal: bool):
    if causal not in _NC_CACHE:
        _NC_CACHE[causal] = build_nc(causal)
    return _NC_CACHE[causal]


def _make_cmask():
    # cmask[p, u] = 1.0 iff p <= u - SB   (slice at s0 = SB + SB*j - 128*t
    # gives keep iff 128t+p <= 512j+c)
    p = np.arange(128)[:, None]
    u = np.arange(2 * SB)[None, :]
    return (p <= u - SB).astype(np.float32)


def make_in_maps(x, wq, bq, wk, bk, wv, bv, wo, bo):
    """Shard full inputs into 8 per-core input maps."""
    f32 = np.float32
    cmask = _make_cmask()
    in_maps = []
    for core in range(NCORES):
        b, hg = divmod(core, NH // HG)
        hs = slice(hg * HD, (hg + 1) * HD)
        wvT = np.zeros((D, VP), f32)
        wvT[:, :HD] = wv[hs, :].T
        bvp = np.zeros((VP,), f32)
        bvp[:HD] = bv[hs]
        in_maps.append(
            {
                "xT": np.ascontiguousarray(x[b].T, f32),
                "wqT": np.ascontiguousarray(wq[hs, :].T, f32),
                "wkT": np.ascontiguousarray(wk[hs, :].T, f32),
                "wvT": wvT,
                "woT": np.ascontiguousarray(wo[:, hs].T, f32),
                "bq": np.ascontiguousarray(bq[hs], f32),
                "bk": np.ascontiguousarray(bk[hs], f32),
                "bv": bvp,
                "cmask": cmask,
            }
        )
    return in_maps


def combine_outputs(results, bo):
    """Sum head-group partials per batch, transpose, add output bias."""
    y = np.empty((B, S, D), np.float32)
    ng = NH // HG
    for b in range(B):
        acc = results[b * ng]["yT"].astype(np.float32)
        for g in range(1, ng):
            acc = acc + results[b * ng + g]["yT"]
        y[b] = acc.T + np.asarray(bo, np.float32)[None, :]
    return y


def kernel(x, wq, bq, wk, bk, wv, bv, wo, bo, mask, _trace=False):
    from concourse.bass_utils import run_bass_kernel_spmd

    causal = bool(np.asarray(mask).item())
    nc = get_nc(causal)
    in_maps = make_in_maps(x, wq, bq, wk, bk, wv, bv, wo, bo)
    res = run_bass_kernel_spmd(nc, in_maps, list(range(NCORES)), trace=_trace)
    y = combine_outputs(res.results, bo)
    if _trace:
        return y, res
    return y


# revision 38
# speedup vs baseline: 1.1340x; 1.1340x over previous
"""Trainium2 Bass kernel for 12-head causal MHA (B=2, S=2048, D=768), fp32.

Sharding: 8 cores = (batch b in {0,1}) x (head-group hg in {0..3}, 3 heads each).
Each core computes, for its (b, hg):
    qT/kT = (x wq_hg^T)^T  (transposed layout, [192, S])
    v     = x wv_hg^T      (natural layout, + ones column for softmax denom)
    flash-style causal attention without max-subtraction (scores are O(1))
    partial yT = wo_hg^T @ outT   ([768, S], row-parallel partial)
Host sums the 4 head-group partials per batch, transposes, adds bo.

Matmul operands live in SBUF as float32r (fp32 bits; 1 cycle/row at N>=256).
The causal mask is a multiplicative 0/1 mask sliced from a [128, 1024]
sliding-window matrix (host input), applied only on diagonal-band tiles.
"""

import math
from contextlib import ExitStack

import numpy as np

import concourse.bacc as bacc
import concourse.bass as bass
import concourse.mybir as mybir
import concourse.tile as tile

FP32 = mybir.dt.float32
FP32R = mybir.dt.float32r

B = 2
S = 2048
D = 768
NH = 12
DK = 64
NCORES = 8
HG = 3  # heads per core
HD = HG * DK  # 192
VP = 256  # padded v width (>=256 keeps float32r at full rate)
KC = D // 128  # 6 contraction chunks of 128
SB = 512  # sequence block (matmul N)
NJ = S // SB  # 4
NT = S // 128  # 16 key tiles
SCALE = 1.0 / math.sqrt(DK)
PSUM_BUFS = (2, 2, 2, 2)  # proj, sp (scores), op (attn out), yp (y proj)


def _r(ap):
    """float32r view of an fp32 DRAM AP (same bytes) for DMA into fp32r SBUF."""
    return ap.bitcast(FP32R)


def build_nc(causal: bool):
    nc = bacc.Bacc(trn_type="TRN2", target_bir_lowering=False, debug=False)

    xT_d = nc.declare_dram_parameter("xT", [D, S], FP32, isOutput=False)
    wqT_d = nc.declare_dram_parameter("wqT", [D, HD], FP32, isOutput=False)
    wkT_d = nc.declare_dram_parameter("wkT", [D, HD], FP32, isOutput=False)
    wvT_d = nc.declare_dram_parameter("wvT", [D, VP], FP32, isOutput=False)
    woT_d = nc.declare_dram_parameter("woT", [HD, D], FP32, isOutput=False)
    bq_d = nc.declare_dram_parameter("bq", [HD], FP32, isOutput=False)
    bk_d = nc.declare_dram_parameter("bk", [HD], FP32, isOutput=False)
    bv_d = nc.declare_dram_parameter("bv", [VP], FP32, isOutput=False)
    cm_d = nc.declare_dram_parameter("cmask", [128, 2 * SB], FP32, isOutput=False)
    yT_d = nc.declare_dram_parameter("yT", [D, S], FP32, isOutput=True)

    EXP = mybir.ActivationFunctionType.Exp

    with tile.TileContext(nc) as tc, ExitStack() as ctx:
        consts = ctx.enter_context(tc.tile_pool(name="consts", bufs=1))

        # ---- constant / persistent SBUF tensors ----
        xT_sb = consts.tile([128, KC, S], FP32R)  # x^T, chunk c = rows 128c..
        wqT_sb = consts.tile([128, KC, HD], FP32R)
        wkT_sb = consts.tile([128, KC, HD], FP32R)
        wvT_sb = consts.tile([128, KC, VP], FP32R)
        woT_sb = [consts.tile([64, D], FP32R, name=f"woT{h}") for h in range(HG)]
        bq0_sb = consts.tile([128, 1], FP32, name="bq0")
        bq1_sb = consts.tile([64, 1], FP32, name="bq1")
        bk0_sb = consts.tile([128, 1], FP32, name="bk0")
        bk1_sb = consts.tile([64, 1], FP32, name="bk1")
        bvb_sb = consts.tile([128, VP], FP32)  # bv broadcast to all partitions
        v65_sb = consts.tile([128, NT, HG, 65], FP32R)  # v tiles + ones column
        qT01_sb = consts.tile([128, S], FP32R)  # q^T heads 0,1
        qT2_sb = consts.tile([64, S], FP32R)  # q^T head 2
        kT01_sb = consts.tile([128, S], FP32R)
        kT2_sb = consts.tile([64, S], FP32R)
        if causal:
            cm_sb = consts.tile([128, 2 * SB], FP32)
            nc.sync.dma_start(out=cm_sb, in_=cm_d.ap())

        # v-projection weights first, then x column-block by column-block so
        # the v projection can start after ~1.5MB instead of the full 8.7MB.
        nc.sync.dma_start(
            out=wvT_sb, in_=_r(wvT_d.ap().rearrange("(c p) n -> p c n", p=128))
        )
        xT_r = xT_d.ap().rearrange("(c p) s -> p c s", p=128)

        def load_x_block(jb, split=False):
            for c in range(KC):
                eng = nc.gpsimd if split and c >= KC // 2 else nc.sync
                eng.dma_start(
                    out=xT_sb[:, c, jb * SB : (jb + 1) * SB],
                    in_=_r(xT_r[:, c, jb * SB : (jb + 1) * SB]),
                )

        # tiny constants first so nothing downstream waits on them
        nc.sync.dma_start(
            out=bq0_sb, in_=bq_d.ap()[0:128].rearrange("(p o) -> p o", o=1)
        )
        nc.sync.dma_start(
            out=bq1_sb, in_=bq_d.ap()[128:192].rearrange("(p o) -> p o", o=1)
        )
        nc.sync.dma_start(
            out=bk0_sb, in_=bk_d.ap()[0:128].rearrange("(p o) -> p o", o=1)
        )
        nc.sync.dma_start(
            out=bk1_sb, in_=bk_d.ap()[128:192].rearrange("(p o) -> p o", o=1)
        )
        # broadcast bv across all 128 partitions with a step-0 partition AP
        bv_ap = bv_d.ap()
        bvb_src = bass.AP(
            tensor=bv_ap.tensor, offset=bv_ap.offset, ap=[[0, 128], [1, VP]]
        )
        nc.sync.dma_start(out=bvb_sb, in_=bvb_src)

        # x block 0 on the Pool queue, q/k weights on SP — both land ~6us in
        # so the first attention block starts early. Outputs + rz hops also
        # use the Pool queue so they don't wait behind bulk input loads.
        for c in range(KC):
            nc.gpsimd.dma_start(
                out=xT_sb[:, c, 0:SB], in_=_r(xT_r[:, c, 0:SB])
            )
        nc.sync.dma_start(
            out=wqT_sb, in_=_r(wqT_d.ap().rearrange("(c p) n -> p c n", p=128))
        )
        nc.sync.dma_start(
            out=wkT_sb, in_=_r(wkT_d.ap().rearrange("(c p) n -> p c n", p=128))
        )
        load_x_block(1)
        for h in range(HG):
            nc.gpsimd.dma_start(
                out=woT_sb[h], in_=_r(woT_d.ap()[h * 64 : (h + 1) * 64, :])
            )
        load_x_block(2)
        load_x_block(3)

        nc.vector.memset(v65_sb.bitcast(FP32), 1.0)  # preset ones column

        # One fused per-block pipeline: for each 512-column sequence block,
        # project v/q/k for that block, then run attention + output
        # projection. Each block only depends on x columns loaded so far, so
        # compute streams behind the DMA.
        proj_pool = ctx.enter_context(
            tc.tile_pool(name="proj", bufs=PSUM_BUFS[0], space="PSUM")
        )
        sp_pool = ctx.enter_context(
            tc.tile_pool(name="sp", bufs=PSUM_BUFS[1], space="PSUM")
        )
        op_pool = ctx.enter_context(
            tc.tile_pool(name="op", bufs=PSUM_BUFS[2], space="PSUM")
        )
        yp_pool = ctx.enter_context(
            tc.tile_pool(name="yp", bufs=PSUM_BUFS[3], space="PSUM")
        )
        et_pool = ctx.enter_context(tc.tile_pool(name="et", bufs=3))
        ef_pool = ctx.enter_context(tc.tile_pool(name="ef", bufs=2))
        rc_pool = ctx.enter_context(tc.tile_pool(name="rc", bufs=2))
        ot_pool = ctx.enter_context(tc.tile_pool(name="ot", bufs=6))
        yt_pool = ctx.enter_context(tc.tile_pool(name="yt", bufs=3))

        def project_block(j):
            # v projection for this block's 4 key tiles (x^T stationary)
            for st in range(4 * j, 4 * (j + 1)):
                vp = proj_pool.tile([128, VP], FP32, name="vp", tag="proj")
                for c in range(KC):
                    nc.tensor.matmul(
                        vp,
                        lhsT=xT_sb[:, c, st * 128 : (st + 1) * 128],
                        rhs=wvT_sb[:, c, :],
                        start=(c == 0),
                        stop=(c == KC - 1),
                    )
                for h in range(HG):
                    nc.vector.tensor_add(
                        v65_sb[:, st, h, 0:64],
                        vp[:, h * 64 : (h + 1) * 64],
                        bvb_sb[:, h * 64 : (h + 1) * 64],
                    )

            # q/k projections for this block (w stationary, transposed out)
            for w_sb, b0, b1, dst01, dst2 in (
                (wqT_sb, bq0_sb, bq1_sb, qT01_sb, qT2_sb),
                (wkT_sb, bk0_sb, bk1_sb, kT01_sb, kT2_sb),
            ):
                for mt, m, dst, bias in ((0, 128, dst01, b0), (1, 64, dst2, b1)):
                    pp = proj_pool.tile([128, SB], FP32, name="pp", tag="proj")
                    for c in range(KC):
                        nc.tensor.matmul(
                            pp[0:m, :],
                            lhsT=w_sb[:, c, mt * 128 : mt * 128 + m],
                            rhs=xT_sb[:, c, j * SB : (j + 1) * SB],
                            start=(c == 0),
                            stop=(c == KC - 1),
                        )
                    nc.vector.tensor_scalar_add(
                        dst[0:m, j * SB : (j + 1) * SB], pp[0:m, :], bias[0:m, :]
                    )

        def attend_block(j):
            out_tiles = []
            for h in range(HG):
                if h < 2:
                    qsrc, ksrc, base = qT01_sb, kT01_sb, 64 * h
                else:
                    qsrc, ksrc, base = qT2_sb, kT2_sb, 0
                tend = 4 * (j + 1) if causal else NT
                ndiag = tend - 4 * j if causal else 0  # trailing diagonal tiles
                nfull = tend - ndiag
                op = op_pool.tile([65, SB], FP32)

                def scores(dst, t, off=0):
                    nc.tensor.matmul(
                        dst,
                        lhsT=ksrc[base : base + 64, t * 128 : (t + 1) * 128],
                        rhs=qsrc[base : base + 64, j * SB + off : (j + 1) * SB],
                        start=True,
                        stop=True,
                    )

                def attnv(t, et_ap, off=0):
                    nc.tensor.matmul(
                        op[:, off:SB],
                        lhsT=v65_sb[:, t, h, :],
                        rhs=et_ap,
                        start=(t == 0),
                        stop=(t == tend - 1),
                    )

                # full (off-diagonal) tiles
                for t in range(nfull):
                    sp = sp_pool.tile([128, SB], FP32)
                    scores(sp, t)
                    et = et_pool.tile([128, SB], FP32R)
                    nc.scalar.activation(et, sp, EXP, scale=SCALE)
                    attnv(t, et)
                # diagonal tiles: trim to useful causal width, exp, then
                # multiply by the 0/1 mask (keep iff p <= c_local)
                for t in range(nfull, tend):
                    off = 128 * t - SB * j
                    n = SB - off
                    sp = sp_pool.tile([128, SB], FP32)
                    scores(sp[:, 0:n], t, off)
                    et = et_pool.tile([128, SB], FP32R)
                    ef = ef_pool.tile([128, SB], FP32)
                    nc.scalar.activation(ef[:, 0:n], sp[:, 0:n], EXP, scale=SCALE)
                    nc.vector.tensor_mul(et[:, 0:n], ef[:, 0:n], cm_sb[:, SB : SB + n])
                    attnv(t, et[:, 0:n], off)
                # normalize: rows 0:64 / row 64 (gpsimd partition broadcast).
                # partition_broadcast HW ucode reads partition 0 regardless of
                # the AP offset, so DMA-hop the reciprocal row to partition 0.
                rc = rc_pool.tile([65, SB], FP32)
                nc.vector.reciprocal(rc[64:65, :], op[64:65, :])
                rz = rc_pool.tile([1, SB], FP32, name="rz")
                nc.gpsimd.dma_start(out=rz, in_=rc[64:65, :])
                bc = rc_pool.tile([64, SB], FP32, name="bc")
                nc.gpsimd.partition_broadcast(bc, rz[0:1, :])
                ot = ot_pool.tile([64, SB], FP32R)
                nc.vector.tensor_mul(ot, op[0:64, :], bc)
                out_tiles.append(ot)

            for dt in range(KC):
                yp = yp_pool.tile([128, SB], FP32, name="yp")
                for h in range(HG):
                    nc.tensor.matmul(
                        yp,
                        lhsT=woT_sb[h][:, dt * 128 : (dt + 1) * 128],
                        rhs=out_tiles[h],
                        start=(h == 0),
                        stop=(h == HG - 1),
                    )
                yt = yt_pool.tile([128, SB], FP32)
                nc.vector.tensor_copy(yt, yp)
                nc.gpsimd.dma_start(
                    out=yT_d.ap()[dt * 128 : (dt + 1) * 128, j * SB : (j + 1) * SB],
                    in_=yt,
                )

        if causal:
            # fused: attention j only needs k/v tiles t < 4(j+1)
            for j in range(NJ):
                project_block(j)
                attend_block(j)
        else:
            # full attention needs all k/v before any attention block
            for j in range(NJ):
                project_block(j)
            for j in range(NJ):
                attend_block(j)

    nc.finalize()
    return nc


_NC_CACHE: dict[bool, object] = {}


def get_nc(causal: bool):
    if causal not in _NC_CACHE:
        _NC_CACHE[causal] = build_nc(causal)
    return _NC_CACHE[causal]


def _make_cmask():
    # cmask[p, u] = 1.0 iff p <= u - SB   (slice at s0 = SB + SB*j - 128*t
    # gives keep iff 128t+p <= 512j+c)
    p = np.arange(128)[:, None]
    u = np.arange(2 * SB)[None, :]
    return (p <= u - SB).astype(np.float32)


def make_in_maps(x, wq, bq, wk, bk, wv, bv, wo, bo):
    """Shard full inputs into 8 per-core input maps."""
    f32 = np.float32
    cmask = _make_cmask()
    in_maps = []
    for core in range(NCORES):
        b, hg = divmod(core, NH // HG)
        hs = slice(hg * HD, (hg + 1) * HD)
        wvT = np.zeros((D, VP), f32)
        wvT[:, :HD] = wv[hs, :].T
        bvp = np.zeros((VP,), f32)
        bvp[:HD] = bv[hs]
        in_maps.append(
            {
                "xT": np.ascontiguousarray(x[b].T, f32),
                "wqT": np.ascontiguousarray(wq[hs, :].T, f32),
                "wkT": np.ascontiguousarray(wk[hs, :].T, f32),
                "wvT": wvT,
                "woT": np.ascontiguousarray(wo[:, hs].T, f32),
                "bq": np.ascontiguousarray(bq[hs], f32),
                "bk": np.ascontiguousarray(bk[hs], f32),
                "bv": bvp,
                "cmask": cmask,
            }
        )
    return in_maps


def combine_outputs(results, bo):
    """Sum head-group partials per batch, transpose, add output bias."""
    y = np.empty((B, S, D), np.float32)
    ng = NH // HG
    for b in range(B):
        acc = results[b * ng]["yT"].astype(np.float32)
        for g in range(1, ng):
            acc = acc + results[b * ng + g]["yT"]
        y[b] = acc.T + np.asarray(bo, np.float32)[None, :]
    return y


def kernel(x, wq, bq, wk, bk, wv, bv, wo, bo, mask, _trace=False):
    from concourse.bass_utils import run_bass_kernel_spmd

    causal = bool(np.asarray(mask).item())
    nc = get_nc(causal)
    in_maps = make_in_maps(x, wq, bq, wk, bk, wv, bv, wo, bo)
    res = run_bass_kernel_spmd(nc, in_maps, list(range(NCORES)), trace=_trace)
    y = combine_outputs(res.results, bo)
    if _trace:
        return y, res
    return y


# revision 41
# speedup vs baseline: 20243.0089x; 17850.4296x over previous
"""Trainium2 Bass kernel for 12-head causal MHA (B=2, S=2048, D=768), fp32.

Sharding: 8 cores = (batch b in {0,1}) x (head-group hg in {0..3}, 3 heads each).
Each core computes, for its (b, hg):
    qT/kT = (x wq_hg^T)^T  (transposed layout, [192, S])
    v     = x wv_hg^T      (natural layout, + ones column for softmax denom)
    flash-style causal attention without max-subtraction (scores are O(1))
    partial yT = wo_hg^T @ outT   ([768, S], row-parallel partial)
Host sums the 4 head-group partials per batch, transposes, adds bo.

Matmul operands live in SBUF as float32r (fp32 bits; 1 cycle/row at N>=256).
The causal mask is a multiplicative 0/1 mask sliced from a [128, 1024]
sliding-window matrix (host input), applied only on diagonal-band tiles.
"""

import math
from contextlib import ExitStack

import numpy as np

import concourse.bacc as bacc
import concourse.bass as bass
import concourse.mybir as mybir
import concourse.tile as tile

FP32 = mybir.dt.float32
FP32R = mybir.dt.float32r

B = 2
S = 2048
D = 768
NH = 12
DK = 64
NCORES = 8
HG = 3  # heads per core
HD = HG * DK  # 192
VP = 256  # padded v width (>=256 keeps float32r at full rate)
KC = D // 128  # 6 contraction chunks of 128
SB = 512  # sequence block (matmul N)
NJ = S // SB  # 4
NT = S // 128  # 16 key tiles
SCALE = 1.0 / math.sqrt(DK)
PSUM_BUFS = (2, 2, 2, 2)  # proj, sp (scores), op (attn out), yp (y proj)


def _r(ap):
    """float32r view of an fp32 DRAM AP (same bytes) for DMA into fp32r SBUF."""
    return ap.bitcast(FP32R)


def build_nc(causal: bool):
    nc = bacc.Bacc(trn_type="TRN2", target_bir_lowering=False, debug=False)

    xT_d = nc.declare_dram_parameter("xT", [D, S], FP32, isOutput=False)
    wqT_d = nc.declare_dram_parameter("wqT", [D, HD], FP32, isOutput=False)
    wkT_d = nc.declare_dram_parameter("wkT", [D, HD], FP32, isOutput=False)
    wvT_d = nc.declare_dram_parameter("wvT", [D, VP], FP32, isOutput=False)
    woT_d = nc.declare_dram_parameter("woT", [HD, D], FP32, isOutput=False)
    bq_d = nc.declare_dram_parameter("bq", [HD], FP32, isOutput=False)
    bk_d = nc.declare_dram_parameter("bk", [HD], FP32, isOutput=False)
    bv_d = nc.declare_dram_parameter("bv", [VP], FP32, isOutput=False)
    cm_d = nc.declare_dram_parameter("cmask", [128, 2 * SB], FP32, isOutput=False)
    yT_d = nc.declare_dram_parameter("yT", [D, S], FP32, isOutput=True)

    EXP = mybir.ActivationFunctionType.Exp

    with tile.TileContext(nc) as tc, ExitStack() as ctx:
        consts = ctx.enter_context(tc.tile_pool(name="consts", bufs=1))

        # ---- constant / persistent SBUF tensors ----
        xT_sb = consts.tile([128, KC, S], FP32R)  # x^T, chunk c = rows 128c..
        wqT_sb = consts.tile([128, KC, HD], FP32R)
        wkT_sb = consts.tile([128, KC, HD], FP32R)
        wvT_sb = consts.tile([128, KC, VP], FP32R)
        woT_sb = [consts.tile([64, D], FP32R, name=f"woT{h}") for h in range(HG)]
        bq0_sb = consts.tile([128, 1], FP32, name="bq0")
        bq1_sb = consts.tile([64, 1], FP32, name="bq1")
        bk0_sb = consts.tile([128, 1], FP32, name="bk0")
        bk1_sb = consts.tile([64, 1], FP32, name="bk1")
        bvb_sb = consts.tile([128, VP], FP32)  # bv broadcast to all partitions
        v65_sb = consts.tile([128, NT, HG, 65], FP32R)  # v tiles + ones column
        qT01_sb = consts.tile([128, S], FP32R)  # q^T heads 0,1
        qT2_sb = consts.tile([64, S], FP32R)  # q^T head 2
        kT01_sb = consts.tile([128, S], FP32R)
        kT2_sb = consts.tile([64, S], FP32R)
        if causal:
            cm_sb = consts.tile([128, 2 * SB], FP32)
            nc.sync.dma_start(out=cm_sb, in_=cm_d.ap())

        # v-projection weights first, then x column-block by column-block so
        # the v projection can start after ~1.5MB instead of the full 8.7MB.
        wvT_r = wvT_d.ap().rearrange("(c p) n -> p c n", p=128)
        nc.sync.dma_start(out=wvT_sb[:, 0 : KC // 2, :], in_=_r(wvT_r[:, 0 : KC // 2, :]))
        nc.gpsimd.dma_start(out=wvT_sb[:, KC // 2 :, :], in_=_r(wvT_r[:, KC // 2 :, :]))
        xT_r = xT_d.ap().rearrange("(c p) s -> p c s", p=128)

        def load_x_block(jb, split=False):
            for c in range(KC):
                eng = nc.gpsimd if split and c >= KC // 2 else nc.sync
                eng.dma_start(
                    out=xT_sb[:, c, jb * SB : (jb + 1) * SB],
                    in_=_r(xT_r[:, c, jb * SB : (jb + 1) * SB]),
                )

        # tiny constants first so nothing downstream waits on them
        nc.sync.dma_start(
            out=bq0_sb, in_=bq_d.ap()[0:128].rearrange("(p o) -> p o", o=1)
        )
        nc.sync.dma_start(
            out=bq1_sb, in_=bq_d.ap()[128:192].rearrange("(p o) -> p o", o=1)
        )
        nc.sync.dma_start(
            out=bk0_sb, in_=bk_d.ap()[0:128].rearrange("(p o) -> p o", o=1)
        )
        nc.sync.dma_start(
            out=bk1_sb, in_=bk_d.ap()[128:192].rearrange("(p o) -> p o", o=1)
        )
        # broadcast bv across all 128 partitions with a step-0 partition AP
        bv_ap = bv_d.ap()
        bvb_src = bass.AP(
            tensor=bv_ap.tensor, offset=bv_ap.offset, ap=[[0, 128], [1, VP]]
        )
        nc.sync.dma_start(out=bvb_sb, in_=bvb_src)

        # x block 0 on the Pool queue, q/k weights on SP — both land ~6us in
        # so the first attention block starts early. Outputs + rz hops also
        # use the Pool queue so they don't wait behind bulk input loads.
        for c in range(KC):
            nc.gpsimd.dma_start(
                out=xT_sb[:, c, 0:SB], in_=_r(xT_r[:, c, 0:SB])
            )
        nc.sync.dma_start(
            out=wqT_sb, in_=_r(wqT_d.ap().rearrange("(c p) n -> p c n", p=128))
        )
        nc.sync.dma_start(
            out=wkT_sb, in_=_r(wkT_d.ap().rearrange("(c p) n -> p c n", p=128))
        )
        load_x_block(1)
        for h in range(HG):
            nc.gpsimd.dma_start(
                out=woT_sb[h], in_=_r(woT_d.ap()[h * 64 : (h + 1) * 64, :])
            )
        load_x_block(2)
        load_x_block(3)

        nc.vector.memset(v65_sb.bitcast(FP32), 1.0)  # preset ones column

        # One fused per-block pipeline: for each 512-column sequence block,
        # project v/q/k for that block, then run attention + output
        # projection. Each block only depends on x columns loaded so far, so
        # compute streams behind the DMA.
        proj_pool = ctx.enter_context(
            tc.tile_pool(name="proj", bufs=PSUM_BUFS[0], space="PSUM")
        )
        sp_pool = ctx.enter_context(
            tc.tile_pool(name="sp", bufs=PSUM_BUFS[1], space="PSUM")
        )
        op_pool = ctx.enter_context(
            tc.tile_pool(name="op", bufs=PSUM_BUFS[2], space="PSUM")
        )
        yp_pool = ctx.enter_context(
            tc.tile_pool(name="yp", bufs=PSUM_BUFS[3], space="PSUM")
        )
        et_pool = ctx.enter_context(tc.tile_pool(name="et", bufs=3))
        ef_pool = ctx.enter_context(tc.tile_pool(name="ef", bufs=2))
        rc_pool = ctx.enter_context(tc.tile_pool(name="rc", bufs=2))
        ot_pool = ctx.enter_context(tc.tile_pool(name="ot", bufs=6))
        yt_pool = ctx.enter_context(tc.tile_pool(name="yt", bufs=3))

        def project_block(j):
            # v projection for this block's 4 key tiles (x^T stationary)
            for st in range(4 * j, 4 * (j + 1)):
                vp = proj_pool.tile([128, VP], FP32, name="vp", tag="proj")
                for c in range(KC):
                    nc.tensor.matmul(
                        vp,
                        lhsT=xT_sb[:, c, st * 128 : (st + 1) * 128],
                        rhs=wvT_sb[:, c, :],
                        start=(c == 0),
                        stop=(c == KC - 1),
                    )
                for h in range(HG):
                    nc.vector.tensor_add(
                        v65_sb[:, st, h, 0:64],
                        vp[:, h * 64 : (h + 1) * 64],
                        bvb_sb[:, h * 64 : (h + 1) * 64],
                    )

            # q/k projections for this block (w stationary, transposed out)
            for w_sb, b0, b1, dst01, dst2 in (
                (wqT_sb, bq0_sb, bq1_sb, qT01_sb, qT2_sb),
                (wkT_sb, bk0_sb, bk1_sb, kT01_sb, kT2_sb),
            ):
                for mt, m, dst, bias in ((0, 128, dst01, b0), (1, 64, dst2, b1)):
                    pp = proj_pool.tile([128, SB], FP32, name="pp", tag="proj")
                    for c in range(KC):
                        nc.tensor.matmul(
                            pp[0:m, :],
                            lhsT=w_sb[:, c, mt * 128 : mt * 128 + m],
                            rhs=xT_sb[:, c, j * SB : (j + 1) * SB],
                            start=(c == 0),
                            stop=(c == KC - 1),
                        )
                    nc.vector.tensor_scalar_add(
                        dst[0:m, j * SB : (j + 1) * SB], pp[0:m, :], bias[0:m, :]
                    )

        def attend_block(j):
            out_tiles = []
            for h in range(HG):
                if h < 2:
                    qsrc, ksrc, base = qT01_sb, kT01_sb, 64 * h
                else:
                    qsrc, ksrc, base = qT2_sb, kT2_sb, 0
                tend = 4 * (j + 1) if causal else NT
                ndiag = tend - 4 * j if causal else 0  # trailing diagonal tiles
                nfull = tend - ndiag
                op = op_pool.tile([65, SB], FP32)

                def scores(dst, t, off=0):
                    nc.tensor.matmul(
                        dst,
                        lhsT=ksrc[base : base + 64, t * 128 : (t + 1) * 128],
                        rhs=qsrc[base : base + 64, j * SB + off : (j + 1) * SB],
                        start=True,
                        stop=True,
                    )

                def attnv(t, et_ap, off=0):
                    nc.tensor.matmul(
                        op[:, off:SB],
                        lhsT=v65_sb[:, t, h, :],
                        rhs=et_ap,
                        start=(t == 0),
                        stop=(t == tend - 1),
                    )

                # full (off-diagonal) tiles
                for t in range(nfull):
                    sp = sp_pool.tile([128, SB], FP32)
                    scores(sp, t)
                    et = et_pool.tile([128, SB], FP32R)
                    nc.scalar.activation(et, sp, EXP, scale=SCALE)
                    attnv(t, et)
                # diagonal tiles: trim to useful causal width, exp, then
                # multiply by the 0/1 mask (keep iff p <= c_local)
                for t in range(nfull, tend):
                    off = 128 * t - SB * j
                    n = SB - off
                    sp = sp_pool.tile([128, SB], FP32)
                    scores(sp[:, 0:n], t, off)
                    et = et_pool.tile([128, SB], FP32R)
                    ef = ef_pool.tile([128, SB], FP32)
                    nc.scalar.activation(ef[:, 0:n], sp[:, 0:n], EXP, scale=SCALE)
                    nc.vector.tensor_mul(et[:, 0:n], ef[:, 0:n], cm_sb[:, SB : SB + n])
                    attnv(t, et[:, 0:n], off)
                # normalize: rows 0:64 / row 64 (gpsimd partition broadcast).
                # partition_broadcast HW ucode reads partition 0 regardless of
                # the AP offset, so DMA-hop the reciprocal row to partition 0.
                rc = rc_pool.tile([65, SB], FP32)
                nc.vector.reciprocal(rc[64:65, :], op[64:65, :])
                rz = rc_pool.tile([1, SB], FP32, name="rz")
                nc.gpsimd.dma_start(out=rz, in_=rc[64:65, :])
                bc = rc_pool.tile([64, SB], FP32, name="bc")
                nc.gpsimd.partition_broadcast(bc, rz[0:1, :])
                ot = ot_pool.tile([64, SB], FP32R)
                nc.vector.tensor_mul(ot, op[0:64, :], bc)
                out_tiles.append(ot)

            for dt in range(KC):
                yp = yp_pool.tile([128, SB], FP32, name="yp")
                for h in range(HG):
                    nc.tensor.matmul(
                        yp,
                        lhsT=woT_sb[h][:, dt * 128 : (dt + 1) * 128],
                        rhs=out_tiles[h],
                        start=(h == 0),
                        stop=(h == HG - 1),
                    )
                yt = yt_pool.tile([128, SB], FP32)
                nc.vector.tensor_copy(yt, yp)
                nc.gpsimd.dma_start(
                    out=yT_d.ap()[dt * 128 : (dt + 1) * 128, j * SB : (j + 1) * SB],
                    in_=yt,
                )

        if causal:
            # fused: attention j only needs k/v tiles t < 4(j+1)
            for j in range(NJ):
                project_block(j)
                attend_block(j)
        else:
            # full attention needs all k/v before any attention block
            for j in range(NJ):
                project_block(j)
            for j in range(NJ):
                attend_block(j)

    nc.finalize()
    return nc


_NC_CACHE: dict[bool, object] = {}


def get_nc(causal: bool):
    if causal not in _NC_CACHE:
        _NC_CACHE[causal] = build_nc(causal)
    return _NC_CACHE[causal]


def _make_cmask():
    # cmask[p, u] = 1.0 iff p <= u - SB   (slice at s0 = SB + SB*j - 128*t
    # gives keep iff 128t+p <= 512j+c)
    p = np.arange(128)[:, None]
    u = np.arange(2 * SB)[None, :]
    return (p <= u - SB).astype(np.float32)


def make_in_maps(x, wq, bq, wk, bk, wv, bv, wo, bo):
    """Shard full inputs into 8 per-core input maps."""
    f32 = np.float32
    cmask = _make_cmask()
    in_maps = []
    for core in range(NCORES):
        b, hg = divmod(core, NH // HG)
        hs = slice(hg * HD, (hg + 1) * HD)
        wvT = np.zeros((D, VP), f32)
        wvT[:, :HD] = wv[hs, :].T
        bvp = np.zeros((VP,), f32)
        bvp[:HD] = bv[hs]
        in_maps.append(
            {
                "xT": np.ascontiguousarray(x[b].T, f32),
                "wqT": np.ascontiguousarray(wq[hs, :].T, f32),
                "wkT": np.ascontiguousarray(wk[hs, :].T, f32),
                "wvT": wvT,
                "woT": np.ascontiguousarray(wo[:, hs].T, f32),
                "bq": np.ascontiguousarray(bq[hs], f32),
                "bk": np.ascontiguousarray(bk[hs], f32),
                "bv": bvp,
                "cmask": cmask,
            }
        )
    return in_maps


def combine_outputs(results, bo):
    """Sum head-group partials per batch, transpose, add output bias."""
    y = np.empty((B, S, D), np.float32)
    ng = NH // HG
    for b in range(B):
        acc = results[b * ng]["yT"].astype(np.float32)
        for g in range(1, ng):
            acc = acc + results[b * ng + g]["yT"]
        y[b] = acc.T + np.asarray(bo, np.float32)[None, :]
    return y


def kernel(x, wq, bq, wk, bk, wv, bv, wo, bo, mask, _trace=False):
    from concourse.bass_utils import run_bass_kernel_spmd

    causal = bool(np.asarray(mask).item())
    nc = get_nc(causal)
    in_maps = make_in_maps(x, wq, bq, wk, bk, wv, bv, wo, bo)
    res = run_bass_kernel_spmd(nc, in_maps, list(range(NCORES)), trace=_trace)
    y = combine_outputs(res.results, bo)
    if _trace:
        return y, res
    return y
